# revision 1
# baseline (speedup 1.0000x reference)
"""Trainium2 Bass kernel for product-key MoE routing (nn_ALL_MOE_59090160058986).

Strategy (8 NeuronCores, token-parallel):
  - Each core owns 256 of the 2048 tokens; weights/keys/tables replicated.
  - BN folded host-side into (wqT, bq). q^T computed on PE (fp32r).
  - scores = q . keys per head: fp32r matmuls into PSUM in 1024-wide chunks
    (2 banks each, triple buffered so PE never waits on a score slot).
  - top-16 per (token, head): per-chunk max8 + max_index on PSUM (DVE);
    candidates packed as (value & ~0x7FF) | within_chunk_idx; a second max8
    stage merges 136 candidates; global indices recovered with float tricks
    on GPSIMD.
  - w_down/w_up fetched with GPSIMD ap_gather from an interleaved table
    in DRAM (64B rows).
  - tiny SwiGLU over the 16 selected experts + softmax gates -> comb.
  - big SwiGLU over x (fp32r) + comb broadcast-add -> out slice [256, 768].
Host assembles the 8 slices into (1, 2048, 768).

Schedule notes (vs. the earlier 597us version):
  - s1/s3/s2 weight DMAs deferred until head 1 so startup DMA feeds the
    selection pipeline first (first max8 ~13us instead of ~31us).
  - keys stream on the sync DMA queue; small gather/scratch DMAs on the
    scalar queue so neither blocks the other.
  - per-(h,t) tail (post-gather tiny swiglu) is drained inside the NEXT
    head's chunk loop so gather latency never blocks the DVE queue head.
  - softmax exp/sum moved to ACT accum_out; gate normalization folded
    into the gw multiply (scalar_tensor_tensor with 1/esum as scalar).
  - down-projection matmuls are issued before the last head's tails
    complete (only the final bias-add waits on comb).
"""
import sys
import numpy as np

sys.path.insert(0, "/opt/trn_rl_repo")

from concourse import bacc, mybir, tile  # noqa: E402
from concourse.bass_utils import run_bass_kernel_spmd  # noqa: E402

dt = mybir.dt

P = 128
B, T, D = 1, 2048, 768
H, N, KD = 4, 25600, 128
KNN = 16
HID_A, HID_S = 44, 1024
NCORES = 8
TC = T // NCORES            # 256 tokens per core
TT = TC // P                # 2 token tiles per core
NCHUNK = 1024               # selection chunk width (2 PSUM banks)
NCH = 25                    # 25 x 1024 = 25600
CW = [NCHUNK] * NCH                      # per-chunk widths
CB = [NCHUNK * c for c in range(NCH)]    # per-chunk base offsets
CAND = NCH * 8              # 136 candidates per row
DK = D // P                 # 6 contraction tiles over D
HT = HID_S // P             # 8 hidden tiles for big swiglu
HK = H * KNN                # 64

MASK_HI = 0xFFFFF800
MASK_LO = 0x000007FF
AF = mybir.ActivationFunctionType
AL = mybir.AluOpType


def round_fp32r(a):
    b = np.ascontiguousarray(a, np.float32).view(np.uint32)
    r = (b.astype(np.uint64) + 0x7FF + ((b >> 12) & 1)) & 0xFFFFF000
    return r.astype(np.uint32).view(np.float32)


def build():
    nc = bacc.Bacc("TRN2", target_bir_lowering=False, debug=False,
                   num_devices=NCORES)

    xTr_d = nc.dram_tensor("xTr", [D, TC], dt.float32r, kind="ExternalInput")
    wqT_d = nc.dram_tensor("wqT", [D, H * KD], dt.float32r, kind="ExternalInput")
    bq_d = nc.dram_tensor("bqf", [P, H], dt.float32, kind="ExternalInput")
    keysT_d = nc.dram_tensor("keysT", [H, KD, N], dt.float32r, kind="ExternalInput")
    tab_d = nc.dram_tensor("tabf", [N, 64], dt.float32, kind="ExternalInput")
    s1T_d = nc.dram_tensor("s1T", [D, HID_S], dt.float32r, kind="ExternalInput")
    s3T_d = nc.dram_tensor("s3T", [D, HID_S], dt.float32r, kind="ExternalInput")
    s2T_d = nc.dram_tensor("s2T", [HID_S, D], dt.float32r, kind="ExternalInput")
    aw1_d = nc.dram_tensor("aw1", [KNN, 2 * HID_A], dt.float32, kind="ExternalInput")
    aw2_d = nc.dram_tensor("aw2", [HID_A, KNN], dt.float32, kind="ExternalInput")
    S_d = nc.dram_tensor("Scol", [P, TT], dt.float32, kind="ExternalInput")
    ones_d = nc.dram_tensor("ones64", [HK, 1], dt.float32, kind="ExternalInput")
    ident_d = nc.dram_tensor("ident", [P, P], dt.float32, kind="ExternalInput")
    cst_d = nc.dram_tensor("csts", [P, 8], dt.uint32, kind="ExternalInput")
    out_d = nc.dram_tensor("out", [TC, D], dt.float32, kind="ExternalOutput")

    with tile.TileContext(nc) as tc:
        with tc.tile_pool(name="wt", bufs=1) as wt, \
             tc.tile_pool(name="keys", bufs=6) as kp, \
             tc.tile_pool(name="selp", bufs=2) as selp, \
             tc.tile_pool(name="persist", bufs=1) as pst, \
             tc.tile_pool(name="smp", bufs=2) as smp, \
             tc.tile_pool(name="drp", bufs=2, space="DRAM") as drp, \
             tc.tile_pool(name="psc", bufs=3, space="PSUM") as psc, \
             tc.tile_pool(name="ptl", bufs=2, space="PSUM") as ptl:

            # ---------------- resident weights / constants ----------------
            xTr = wt.tile([P, DK, TC], dt.float32r)
            wqT = wt.tile([P, DK, H * KD], dt.float32r)
            bqc = wt.tile([P, H], dt.float32)
            Scol = wt.tile([P, TT], dt.float32)
            ones64 = wt.tile([HK, 1], dt.float32)
            ident = wt.tile([P, P], dt.float32)
            csts = wt.tile([P, 8], dt.uint32)
            aw1 = wt.tile([KNN, 2 * HID_A], dt.float32)
            aw2 = wt.tile([HID_A, KNN], dt.float32)

            # critical-path weights on the sync queue: consts + head-0
            # q-projection slices first so qT(h0) is ready ASAP
            nc.sync.dma_start(out=bqc[:], in_=bq_d[:])
            nc.sync.dma_start(out=csts[:], in_=cst_d[:])
            for k in range(DK):
                nc.sync.dma_start(out=xTr[:, k, :], in_=xTr_d[k * P:(k + 1) * P, :])
                nc.sync.dma_start(out=wqT[:, k, 0:P],
                                  in_=wqT_d[k * P:(k + 1) * P, 0:P])
            for k in range(DK):
                nc.sync.dma_start(out=wqT[:, k, P:H * KD],
                                  in_=wqT_d[k * P:(k + 1) * P, P:H * KD])
            nc.sync.dma_start(out=Scol[:], in_=S_d[:])
            nc.sync.dma_start(out=ones64[:], in_=ones_d[:])
            nc.sync.dma_start(out=ident[:], in_=ident_d[:])
            nc.sync.dma_start(out=aw1[:], in_=aw1_d[:])
            nc.sync.dma_start(out=aw2[:], in_=aw2_d[:])
            # keys prefetch on the (otherwise idle at startup) scalar queue
            NPRE = 6
            kpre = [kp.tile([P, CW[c]], dt.float32r, tag="keys",
                            name=f"kpre{c}") for c in range(NPRE)]
            for c, kt_ in enumerate(kpre):
                nc.scalar.dma_start(out=kt_[:],
                                    in_=keysT_d[0, :, CB[c]:CB[c] + CW[c]])

            c_mhi = csts[:, 0:1]
            c_mlo = csts[:, 1:2]

            # ---------------- q^T : [H*KD, TC] fp32r ----------------
            qT = wt.tile([P, H, TC], dt.float32r)

            def make_qT(h):
                ps = ptl.tile([P, TC], dt.float32, tag="tl", name=f"qps{h}")
                for k in range(DK):
                    nc.tensor.matmul(out=ps[:],
                                     lhsT=wqT[:, k, h * P:(h + 1) * P],
                                     rhs=xTr[:, k, :],
                                     start=(k == 0), stop=(k == DK - 1))
                nc.scalar.activation(out=qT[:, h, :], in_=ps[:],
                                     func=AF.Identity, bias=bqc[:, h:h + 1])

            make_qT(0)   # heads 1-3 are emitted after the first chunk

            # big swiglu weights (loaded later, used from head>=1)
            s13 = wt.tile([P, DK, 2 * HID_S], dt.float32r)
            s2a = wt.tile([P, HT, D], dt.float32r)
            hTr = wt.tile([P, HT, TC], dt.float32r)

            def load_big_weights():
                for k in range(DK):
                    nc.scalar.dma_start(out=s13[:, k, 0:HID_S],
                                        in_=s1T_d[k * P:(k + 1) * P, :])
                    nc.scalar.dma_start(out=s13[:, k, HID_S:2 * HID_S],
                                        in_=s3T_d[k * P:(k + 1) * P, :])
                for ht in range(HT):
                    nc.scalar.dma_start(out=s2a[:, ht, :],
                                        in_=s2T_d[ht * P:(ht + 1) * P, :])

            def big_swiglu(HTA, HTB):
                for ht in range(HTA, HTB):
                    po1 = ptl.tile([P, TC], dt.float32, tag="tl", name=f"bo1_{ht}")
                    po3 = ptl.tile([P, TC], dt.float32, tag="tl", name=f"bo3_{ht}")
                    for k in range(DK):
                        nc.tensor.matmul(out=po1[:],
                                         lhsT=s13[:, k, ht * P:(ht + 1) * P],
                                         rhs=xTr[:, k, :],
                                         start=(k == 0), stop=(k == DK - 1))
                        nc.tensor.matmul(out=po3[:],
                                         lhsT=s13[:, k, HID_S + ht * P:HID_S + (ht + 1) * P],
                                         rhs=xTr[:, k, :],
                                         start=(k == 0), stop=(k == DK - 1))
                    sil = smp.tile([P, TC], dt.float32, tag="bsil")
                    nc.scalar.activation(out=sil[:], in_=po1[:], func=AF.Silu)
                    hf = smp.tile([P, TC], dt.float32, tag="bh")
                    nc.vector.tensor_tensor(out=hf[:], in0=sil[:], in1=po3[:],
                                            op=AL.mult)
                    nc.scalar.activation(out=hTr[:, ht, :], in_=hf[:], func=AF.Copy)

            # ---------------- selection + per-head tail ----------------
            vq_t = [pst.tile([P, HK], dt.float32, tag=f"vq{t}", name=f"vq{t}")
                    for t in range(TT)]
            u_hts = {(hh, t): pst.tile([KNN, P], dt.float32, tag=f"u{hh}_{t}",
                                       name=f"u{hh}_{t}")
                     for hh in range(H) for t in range(TT)}

            deferred = []

            def drain_tail():
                if deferred:
                    deferred.pop(0)()

            for h in range(H):
                cv = [selp.tile([P, CAND], dt.float32, tag=f"cv{t}",
                                name=f"cv{t}_{h}") for t in range(TT)]
                ci = [selp.tile([P, CAND], dt.uint32, tag=f"ci{t}",
                                name=f"ci{t}_{h}") for t in range(TT)]
                for c in range(NCH):
                    w = CW[c]
                    if h == 0 and c < NPRE:
                        kt = kpre[c]
                    else:
                        kt = kp.tile([P, w], dt.float32r, tag="keys")
                        nc.sync.dma_start(out=kt[:],
                                          in_=keysT_d[h, :, CB[c]:CB[c] + w])
                    for t in range(TT):
                        ps = psc.tile([P, w], dt.float32, tag="sc")
                        for j in range(w // 512):
                            nc.tensor.matmul(out=ps[:, j * 512:(j + 1) * 512],
                                             lhsT=qT[:, h, t * P:(t + 1) * P],
                                             rhs=kt[:, j * 512:(j + 1) * 512],
                                             start=True, stop=True)
                        nc.vector.max(out=cv[t][:, c * 8:(c + 1) * 8], in_=ps[:])
                        nc.vector.max_index(out=ci[t][:, c * 8:(c + 1) * 8],
                                            in_max=cv[t][:, c * 8:(c + 1) * 8],
                                            in_values=ps[:])
                    if h == 0 and c == 0:
                        for hh in range(1, H):
                            make_qT(hh)
                    if c == 5 or c == 11:
                        drain_tail()   # previous head's post-gather work

                if h == 0:
                    load_big_weights()

                for t in range(TT):
                    # ---- stage 2: merge packed candidates -> top-16 ----
                    cp = smp.tile([P, CAND], dt.uint32, tag="cp", name=f"cp_{h}_{t}")
                    nc.vector.scalar_tensor_tensor(
                        out=cp[:], in0=cv[t][:].bitcast(dt.uint32), scalar=c_mhi,
                        in1=ci[t][:], op0=AL.bitwise_and, op1=AL.bitwise_or)
                    cpf = cp[:].bitcast(dt.float32)
                    pk = smp.tile([P, KNN], dt.float32, tag="pk", name=f"pk_{h}_{t}")
                    pos = smp.tile([P, KNN], dt.uint32, tag="pos", name=f"pos_{h}_{t}")
                    nc.vector.max(out=pk[:, 0:8], in_=cpf)
                    nc.vector.max_index(out=pos[:, 0:8], in_max=pk[:, 0:8],
                                        in_values=cpf)
                    cp2 = smp.tile([P, CAND], dt.float32, tag="cp2", name=f"cp2_{h}_{t}")
                    nc.vector.match_replace(out=cp2[:], in_to_replace=pk[:, 0:8],
                                            in_values=cpf, imm_value=-1e30)
                    nc.vector.max(out=pk[:, 8:16], in_=cp2[:])
                    nc.vector.max_index(out=pos[:, 8:16], in_max=pk[:, 8:16],
                                        in_values=cpf)
                    pku = pk[:].bitcast(dt.uint32)
                    vq16 = vq_t[t][:, h * KNN:(h + 1) * KNN]
                    nc.vector.tensor_scalar(
                        out=vq16.bitcast(dt.uint32),
                        in0=pku, scalar1=c_mhi, scalar2=None, op0=AL.bitwise_and)
                    i10f = smp.tile([P, KNN], dt.float32, tag="i10f", name=f"i10f_{h}_{t}")
                    i10 = smp.tile([P, KNN], dt.uint32, tag="i10", name=f"i10_{h}_{t}")
                    nc.vector.tensor_scalar(out=i10[:], in0=pku, scalar1=c_mlo,
                                            scalar2=None, op0=AL.bitwise_and)
                    nc.vector.tensor_copy(out=i10f[:], in_=i10[:])
                    posf = smp.tile([P, KNN], dt.float32, tag="posf", name=f"posf_{h}_{t}")
                    nc.vector.tensor_copy(out=posf[:], in_=pos[:])
                    chkf = smp.tile([P, KNN], dt.float32, tag="chkf", name=f"chkf_{h}_{t}")
                    nc.vector.tensor_scalar(out=chkf[:], in0=posf[:],
                                            scalar1=0.125, scalar2=0.5625,
                                            op0=AL.mult, op1=AL.add)
                    nc.vector.tensor_scalar(out=chkf[:], in0=chkf[:],
                                            scalar1=8388608.0, scalar2=8388609.0,
                                            op0=AL.add, op1=AL.subtract)
                    gif = smp.tile([P, KNN], dt.float32, tag="gif", name=f"gif_{h}_{t}")
                    nc.vector.scalar_tensor_tensor(
                        out=gif[:], in0=chkf[:],
                        scalar=float(NCHUNK), in1=i10f[:], op0=AL.mult, op1=AL.add)
                    gic = smp.tile([P, KNN], dt.float32, tag="gic", name=f"gic_{h}_{t}")
                    nc.vector.tensor_scalar(out=gic[:], in0=gif[:],
                                            scalar1=float(N - 1), scalar2=0.0,
                                            op0=AL.min, op1=AL.max)
                    gii = smp.tile([P, KNN], dt.int32, tag="gii", name=f"gii_{h}_{t}")
                    nc.vector.tensor_copy(out=gii[:], in_=gic[:])
                    gis = smp.tile([P, KNN], dt.int16, tag="gis", name=f"gis_{h}_{t}")
                    nc.vector.tensor_copy(out=gis[:], in_=gii[:])

                    # ---- gates on ACT: exp + sum in one op ----
                    gates = smp.tile([P, KNN], dt.float32, tag="gates", name=f"gates_{h}_{t}")
                    nv = smp.tile([P, 1], dt.float32, tag="nv", name=f"nv_{h}_{t}")
                    esum = smp.tile([P, 1], dt.float32, tag="esum", name=f"esum_{h}_{t}")
                    nc.scalar.activation(out=nv[:], in_=vq16[:, 0:1],
                                         func=AF.Copy, scale=-1.0)
                    nc.scalar.activation(out=gates[:], in_=vq16,
                                         func=AF.Exp, bias=nv[:],
                                         accum_out=esum[:])
                    erec = smp.tile([P, 1], dt.float32, tag="erec", name=f"erec_{h}_{t}")
                    nc.vector.reciprocal(out=erec[:], in_=esum[:])

                    # ---- gather w_down/w_up for the selected experts ----
                    scratch = drp.tile([P, KNN], dt.int16, tag="scr", name=f"scr_{h}_{t}")
                    nc.scalar.dma_start(out=scratch[:], in_=gis[:])
                    wrapped = smp.tile([P, 8 * KNN], dt.int16, tag="wrapped", name=f"wrapped_{h}_{t}")
                    src = scratch[:].rearrange("(r m) c -> m c r", r=8, m=16)
                    for g in range(8):
                        dstv = wrapped[g * 16:(g + 1) * 16, :].rearrange(
                            "p (c r) -> p c r", c=KNN, r=8)
                        eng = nc.sync if (h == H - 1 and g % 2 == 1) \
                            else nc.scalar
                        eng.dma_start(out=dstv, in_=src)
                    wdu = smp.tile([P, KNN, 64], dt.float32, tag="wdu", name=f"wdu_{h}_{t}")
                    for kk in range(2):
                        nc.gpsimd.dma_gather(
                            out_ap=wdu[:, kk * 8:(kk + 1) * 8, :], in_ap=tab_d[:],
                            idxs_ap=wrapped[:, kk * 64:(kk + 1) * 64],
                            num_idxs=1024, num_idxs_reg=1024, elem_size=64)
                    wd = wdu[:, :, 0:1].rearrange("p a b -> p (a b)")
                    wu = wdu[:, :, 1:2].rearrange("p a b -> p (a b)")

                    def post_gather(t=t, h=h, wd=wd, wu=wu,
                                    gates=gates, erec=erec):
                        # z = wd * S_t ; gw = gates * (1/esum) * wu
                        zg = smp.tile([P, KNN], dt.float32, tag="zg")
                        nc.vector.tensor_scalar(out=zg[:], in0=wd,
                                                scalar1=Scol[:, t:t + 1],
                                                scalar2=None, op0=AL.mult)
                        gw = smp.tile([P, KNN], dt.float32, tag="gw")
                        nc.vector.scalar_tensor_tensor(
                            out=gw[:], in0=gates[:], scalar=erec[:],
                            in1=wu, op0=AL.mult, op1=AL.mult)
                        pz = ptl.tile([KNN, P], dt.float32, tag="tl",
                                      name=f"pz{t}_{h}")
                        nc.tensor.transpose(out=pz[:], in_=zg[:],
                                            identity=ident[:])
                        z_h = smp.tile([KNN, P], dt.float32, tag="z_h")
                        nc.scalar.activation(out=z_h[:], in_=pz[:], func=AF.Copy)
                        pg = ptl.tile([KNN, P], dt.float32, tag="tl",
                                      name=f"pg{t}_{h}")
                        nc.tensor.transpose(out=pg[:], in_=gw[:],
                                            identity=ident[:])
                        gw_h = smp.tile([KNN, P], dt.float32, tag="gw_h")
                        nc.scalar.activation(out=gw_h[:], in_=pg[:], func=AF.Copy)
                        po1 = ptl.tile([HID_A, P], dt.float32, tag="tl",
                                       name=f"po1{t}_{h}")
                        po3 = ptl.tile([HID_A, P], dt.float32, tag="tl",
                                       name=f"po3{t}_{h}")
                        nc.tensor.matmul(out=po1[:], lhsT=aw1[:, 0:HID_A],
                                         rhs=z_h[:], start=True, stop=True)
                        nc.tensor.matmul(out=po3[:], lhsT=aw1[:, HID_A:2 * HID_A],
                                         rhs=z_h[:], start=True, stop=True)
                        sil = smp.tile([HID_A, P], dt.float32, tag="sil")
                        nc.scalar.activation(out=sil[:], in_=po1[:], func=AF.Silu)
                        g1 = smp.tile([HID_A, P], dt.float32, tag="g1")
                        nc.vector.tensor_tensor(out=g1[:], in0=sil[:], in1=po3[:],
                                                op=AL.mult)
                        ph = ptl.tile([KNN, P], dt.float32, tag="tl",
                                      name=f"ph{t}_{h}")
                        nc.tensor.matmul(out=ph[:], lhsT=aw2[:], rhs=g1[:],
                                         start=True, stop=True)
                        nc.vector.tensor_tensor(
                            out=u_hts[(h, t)][:], in0=ph[:],
                            in1=gw_h[:], op=AL.mult)
                    deferred.append(post_gather)

                if h in (1, 2):
                    big_swiglu(4 * (h - 1), 4 * h)

            # ---------------- final phase ----------------
            # down-projection matmuls are independent of comb: issue first
            pouts = []
            for t in range(TT):
                pout = psc.tile([P, D], dt.float32, tag="sc", name=f"pout{t}")
                for ht in range(HT):
                    nc.tensor.matmul(out=pout[:, 0:512],
                                     lhsT=hTr[:, ht, t * P:(t + 1) * P],
                                     rhs=s2a[:, ht, 0:512],
                                     start=(ht == 0), stop=(ht == HT - 1))
                    nc.tensor.matmul(out=pout[:, 512:D],
                                     lhsT=hTr[:, ht, t * P:(t + 1) * P],
                                     rhs=s2a[:, ht, 512:D],
                                     start=(ht == 0), stop=(ht == HT - 1))
                pouts.append(pout)
                drain_tail()   # h=3 tails overlap the down-projection

            while deferred:
                drain_tail()

            # comb columns
            comb_cols = []
            for t in range(TT):
                pcomb = ptl.tile([1, P], dt.float32, tag="tl",
                                 name=f"pcomb{t}")
                for hh in range(H):
                    nc.tensor.matmul(out=pcomb[:], lhsT=ones64[0:KNN, :],
                                     rhs=u_hts[(hh, t)][:],
                                     start=(hh == 0), stop=(hh == H - 1))
                comb_row = smp.tile([1, P], dt.float32, tag="comb_row")
                nc.scalar.activation(out=comb_row[:], in_=pcomb[:],
                                     func=AF.Copy)
                pcombT = ptl.tile([P, 1], dt.float32, tag="tl", name=f"pcombT{t}")
                nc.tensor.transpose(out=pcombT[:], in_=comb_row[:],
                                    identity=ident[0:1, 0:1])
                comb_col = pst.tile([P, 1], dt.float32, tag=f"comb{t}",
                                    name=f"comb{t}")
                nc.scalar.activation(out=comb_col[:], in_=pcombT[:], func=AF.Copy)
                comb_cols.append(comb_col)

            for t in range(TT):
                out_sb = smp.tile([P, D], dt.float32, tag="out_sb")
                nc.scalar.activation(out=out_sb[:], in_=pouts[t][:],
                                     func=AF.Identity, bias=comb_cols[t][:])
                nc.sync.dma_start(out=out_d[t * P:(t + 1) * P, :], in_=out_sb[:])

    nc.compile()
    return nc


_BUILT = None


def _get_built():
    global _BUILT
    if _BUILT is None:
        _BUILT = build()
    return _BUILT


def _prep_inputs(x, wq, bq, bn_gamma, bn_beta, bn_mean, bn_var, keys,
                 w_down_tab, w_up_tab, a_w1, a_w2, a_w3, s_w1, s_w2, s_w3):
    x2 = np.asarray(x, np.float32).reshape(T, D)
    s = (bn_gamma / np.sqrt(bn_var + 1e-5)).astype(np.float32)
    wqf = (np.asarray(wq, np.float32) * s[:, None])
    bqf = ((np.asarray(bq, np.float32) - bn_mean) * s + bn_beta).astype(np.float32)

    xT = round_fp32r(np.ascontiguousarray(x2.T))                    # [768, 2048]
    wqT = round_fp32r(np.ascontiguousarray(wqf.T))                  # [768, 512]
    bqc = np.ascontiguousarray(bqf.reshape(H, P).T)                 # [128, 4]
    keysT = round_fp32r(np.ascontiguousarray(
        np.asarray(keys, np.float32).transpose(0, 2, 1)))           # [4, 128, 25600]
    tabf = np.zeros((N, 64), np.float32)
    tabf[:, 0] = np.asarray(w_down_tab, np.float32)[:, 0]
    tabf[:, 1] = np.asarray(w_up_tab, np.float32)[:, 0]
    s1T = round_fp32r(np.ascontiguousarray(np.asarray(s_w1, np.float32).T))
    s3T = round_fp32r(np.ascontiguousarray(np.asarray(s_w3, np.float32).T))
    s2T = round_fp32r(np.ascontiguousarray(np.asarray(s_w2, np.float32).T))
    aw1 = np.ascontiguousarray(
        np.concatenate([np.asarray(a_w1, np.float32).T,
                        np.asarray(a_w3, np.float32).T], axis=1))   # [16, 88]
    aw2 = np.ascontiguousarray(np.asarray(a_w2, np.float32).T)      # [44, 16]
    S = x2.sum(axis=1).astype(np.float32)                           # [2048]
    ones64 = np.ones((HK, 1), np.float32)
    ident = np.eye(P, dtype=np.float32)
    csts = np.zeros((P, 8), np.uint32)
    csts[:, 0] = MASK_HI
    csts[:, 1] = MASK_LO
    csts[:, 2] = 3
    csts[:, 3] = 10

    in_maps = []
    for c in range(NCORES):
        sl = slice(c * TC, (c + 1) * TC)
        Scol = np.ascontiguousarray(S[sl].reshape(TT, P).T)         # [128, 2]
        in_maps.append({
            "xTr": np.ascontiguousarray(xT[:, sl]),
            "wqT": wqT,
            "bqf": bqc,
            "keysT": keysT,
            "tabf": tabf,
            "s1T": s1T,
            "s3T": s3T,
            "s2T": s2T,
            "aw1": aw1,
            "aw2": aw2,
            "Scol": Scol,
            "ones64": ones64,
            "ident": ident,
            "csts": csts,
        })
    return in_maps


def kernel(**inputs):
    nc = _get_built()
    in_maps = _prep_inputs(**inputs)
    res = run_bass_kernel_spmd(nc, in_maps, core_ids=list(range(NCORES)))
    out = np.concatenate([res.results[c]["out"] for c in range(NCORES)], axis=0)
    return out.reshape(B, T, D).astype(np.float32)


if __name__ == "__main__":
    d = np.load("/root/problem/inputs_cache.npz")
    inp = {k: d[k] for k in d.files}
    out = kernel(**inp)
    print("out", out.shape, float(np.linalg.norm(out)))
    np.save("/root/problem/kernel_out.npy", out)

